# revision 13
# baseline (speedup 1.0000x reference)
"""Trainium2 Bass kernel for nn_AdaptiveEpisodicMemory (scatter_memory).

Computes, for B=4096 queries over an M=65536-slot memory bank:

    scores = q @ K^T + 0.5 * c @ CTX^T + 0.3*exp(-0.1*(1-t))  (masked by used_slots)
    out    = softmax(scores) @ V

Strategy (8 NeuronCores):
  * Unused slots receive -1e9 scores; their softmax weight is exactly 0 in
    fp32, so the host drops them up-front (exact transformation) and pads the
    survivors to a multiple of 8*128. Shapes are chosen per-input at build
    time, so the kernel is correct for any input.
  * The memory bank (keys/contexts/values) is sharded across the 8 cores;
    query/context are replicated. Per core:
        S^T[m, b]  = KC_shard^T.T @ QC^T      (one K=128-padded matmul, bf16)
        P^T[m, b]  = exp(S^T + bias_m)        (ScalarE for most tiles; a bf16
                                               bit-trick exp on VectorE for a
                                               minority, to relieve ScalarE;
                                               bias folds time-decay + mask)
        O^T[65, b] += Vaug_tile.T @ P^T       (Vaug = [V | 1 | 0-pad]; row 64
                                               accumulates the softmax denom)
  * After each 1024-query pass the [65, 1024] partial is ReduceScattered
    (overlapped with the next pass); at the end each core transposes its
    4x128-query shares, divides by the denominator, and writes them out.
    Host work is limited to layout: compaction/sharding/concat/transpose of
    inputs and reassembly of output slices.
"""
import sys

sys.path.insert(0, "/opt/trn_rl_repo")
import math

import ml_dtypes
import numpy as np

from concourse import bass, bass_utils, masks, mybir, tile

B, M, D, CD = 4096, 65536, 64, 32
KDIM = D + CD  # 96: contraction dim of the fused score matmul
KPAD = 128  # padded to 128 so weight loads take the fast path
VAW = 128  # Vaug padded from 65 to 128 columns, same reason
NCORES = 8
BCHUNK = 512
CPP = 2  # batch chunks per pass (exp runs at FD = CPP*BCHUNK)
PASSB = BCHUNK * CPP
NPASS = B // PASSB
SHARDB = PASSB // NCORES  # batch rows each core owns per pass after RS
F32 = mybir.dt.float32
BF16 = mybir.dt.bfloat16
I16 = mybir.dt.int16
TIME_WEIGHT = 0.1
CURRENT_TIME = 1.0
DECAY_COEF = 0.3
NEG_BIG = -1e9
N_WARMUP_MM = 22
# bf16 bit-trick exp: bf16bits(e^x) ~ round(x * 128/ln2 + (127*128 - 5.5))
A_TRICK = 128.0 / math.log(2.0)
B_TRICK = 127.0 * 128.0 - 5.5
N_ACT_TAIL = 8  # trailing m-tiles always on ScalarE (they may hold -1e9 pads)
DVE_STRIDE = 3  # every 3rd eligible m-tile goes to VectorE


def _dve_ks(ntiles: int) -> set:
    return {k for k in range(max(0, ntiles - N_ACT_TAIL)) if k % DVE_STRIDE == 1}


def _split_multi_waits(nc) -> int:
    """This walrus build accepts at most one fused sync-wait per instruction;
    hoist extras into standalone InstEventSemaphore instructions."""
    n_split = 0
    for fn in nc.m.functions:
        for bb in fn.blocks:
            insts = list(bb.instructions)
            out = []
            changed = False
            for inst in insts:
                si = inst.sync_info
                if si is not None and si.on_wait is not None and len(si.on_wait) > 1:
                    waits = list(si.on_wait)
                    for w in waits[:-1]:
                        ev = mybir.InstEventSemaphore(
                            name=f"{inst.name}-wsplit{n_split}",
                            engine=inst.engine,
                            ins=[],
                            outs=[],
                            sync_info=mybir.SyncInfo(on_wait=[w], on_update=[]),
                            bass_nofuse=True,
                        )
                        out.append(ev)
                        n_split += 1
                    inst.sync_info = mybir.SyncInfo(
                        on_wait=[waits[-1]], on_update=list(si.on_update or [])
                    )
                    changed = True
                out.append(inst)
            if changed:
                bb.instructions[:] = out
    return n_split


def _skip_redundant_ldweights(nc) -> int:
    """After scheduling, matmuls whose stationary operand is identical to the
    immediately preceding matmul's can skip the implicit LDWEIGHTS."""
    n = 0
    for fn in nc.m.functions:
        for bb in fn.blocks:
            prev_w = None
            for inst in bb.instructions:
                if not isinstance(inst, mybir.InstMatmult):
                    continue
                if inst.is_transpose:
                    prev_w = None
                    continue
                w = repr(inst.ins[1])
                if prev_w is not None and w == prev_w:
                    inst.ldweights = False
                    n += 1
                prev_w = w
    return n


def _build(m_loc: int):
    """Build the per-core Bass program for a shard of m_loc memory slots."""
    ntiles = m_loc // 128
    dve_ks = _dve_ks(ntiles)
    nc = bass.Bass(trn_type="TRN2", debug=False, num_devices=NCORES)

    # register the decay-exp bias as a const AP (only 0.0/1.0 are built in)
    decay_bias = math.log(DECAY_COEF) - TIME_WEIGHT * CURRENT_TIME
    ct = nc.alloc_sbuf_tensor("const-float32-extra", [128, 1], F32)
    nc.gpsimd.memset(ct.ap(), decay_bias)
    nc.const_aps.aps[(F32, decay_bias)] = ct.ap()
    nc.all_engine_barrier()

    qc_ext = nc.dram_tensor("qc_t", [KPAD, B], BF16, kind="ExternalInput")
    kc_ext = nc.dram_tensor("kc_t", [KPAD, m_loc], BF16, kind="ExternalInput")
    # vaug arrives pre-arranged tile-major: [128, ntiles*VAW]
    va_ext = nc.dram_tensor("vaug", [128, ntiles * VAW], BF16, kind="ExternalInput")
    ts_ext = nc.dram_tensor("tsm", [128, ntiles], F32, kind="ExternalInput")
    mk_ext = nc.dram_tensor("maskf", [128, ntiles], F32, kind="ExternalInput")
    out_ext = nc.dram_tensor("out", [B // NCORES, D], F32, kind="ExternalOutput")

    NP1 = NPASS - 1
    bounce3 = nc.dram_tensor("rs_in3", [NCORES, NP1, D + 1, SHARDB], F32)
    red3 = nc.dram_tensor("rs_out3", [NP1, D + 1, SHARDB], F32)
    bounce1 = nc.dram_tensor("rs_in1", [NCORES, D + 1, SHARDB], F32)
    red1 = nc.dram_tensor("rs_out1", [D + 1, SHARDB], F32)

    with tile.TileContext(nc) as tc:
        with (
            tc.tile_pool(name="big", bufs=1) as big,
            tc.tile_pool(name="small", bufs=1) as small,
            tc.tile_pool(name="pT", bufs=6) as pTp,
            tc.tile_pool(name="osb", bufs=1) as osb,
            tc.tile_pool(name="psS", bufs=3, space="PSUM") as psS,
            tc.tile_pool(name="psO", bufs=1, space="PSUM") as psO,
            tc.tile_pool(name="fin", bufs=2) as fin,
        ):
            # PE warmup: keep TensorE busy from t=0 so HAM reaches 2.4 GHz
            # before the real matmuls start (inputs are still DMAing in).
            wsrc = small.tile([128, 512], BF16)
            nc.vector.memset(wsrc[:], 1.0)
            wps = psS.tile(
                [128, 512], F32, name="wps", tag="sps", padded_shape=[128, CPP * BCHUNK]
            )
            for _ in range(N_WARMUP_MM):
                nc.tensor.matmul(
                    wps[:], lhsT=wsrc[:, 0:128], rhs=wsrc[:], start=True, stop=True
                )

            # small inputs first so the bias chain is ready early
            ts_s = small.tile([128, ntiles], F32)
            nc.sync.dma_start(ts_s[:], ts_ext.ap())
            mk_s = small.tile([128, ntiles], F32)
            nc.sync.dma_start(mk_s[:], mk_ext.ap())

            # big inputs, chunked; the first pieces of qc/kc come first so the
            # loop can start while the rest streams in
            qc_s = big.tile([KPAD, B], BF16)
            kc_s = big.tile([KPAD, m_loc], BF16)
            va_s = big.tile([128, ntiles * VAW], BF16)
            nkc = min(4, ntiles)
            wq, wk = B // NPASS, m_loc // nkc
            nc.sync.dma_start(qc_s[:, 0:wq], qc_ext.ap()[:, 0:wq])
            nc.sync.dma_start(kc_s[:, 0:wk], kc_ext.ap()[:, 0:wk])
            for c in range(1, nkc):
                nc.sync.dma_start(
                    kc_s[:, c * wk : (c + 1) * wk],
                    kc_ext.ap()[:, c * wk : (c + 1) * wk],
                )
            for c in range(1, NPASS):
                nc.sync.dma_start(
                    qc_s[:, c * wq : (c + 1) * wq],
                    qc_ext.ap()[:, c * wq : (c + 1) * wq],
                )
            for c in range(2):
                h = ntiles * VAW // 2
                nc.sync.dma_start(
                    va_s[:, c * h : (c + 1) * h], va_ext.ap()[:, c * h : (c + 1) * h]
                )

            # identity for the epilogue transposes (GpSimd, overlaps DMAs)
            ident = small.tile([128, 128], F32)
            masks.make_identity(nc, ident[:])

            # bias_m = 0.3*exp(0.1*t - 0.1) + (mask ? 0 : -1e9)
            #        = exp(0.1*t + (ln 0.3 - 0.1)) + (mask - 1) * 1e9
            d_s = small.tile([128, ntiles], F32)
            nc.scalar.activation(
                d_s[:],
                ts_s[:],
                mybir.ActivationFunctionType.Exp,
                bias=decay_bias,
                scale=TIME_WEIGHT,
            )
            mneg = small.tile([128, ntiles], F32)
            nc.vector.tensor_scalar(
                mneg[:],
                mk_s[:],
                -NEG_BIG,
                NEG_BIG,
                mybir.AluOpType.mult,
                mybir.AluOpType.add,
            )
            bias_s = small.tile([128, ntiles], F32)
            nc.vector.tensor_add(bias_s[:], d_s[:], mneg[:])
            # for the VectorE trick-exp tiles: bias2 = bias * A + B
            bias2_s = small.tile([128, ntiles], F32)
            nc.vector.tensor_scalar(
                bias2_s[:],
                bias_s[:],
                A_TRICK,
                B_TRICK,
                mybir.AluOpType.mult,
                mybir.AluOpType.add,
            )

            oall = osb.tile([D + 1, B], F32)

            for p in range(NPASS):
                oaccs = [
                    psO.tile([128, BCHUNK], F32, name=f"oacc{i}", tag=f"oacc{i}")
                    for i in range(CPP)
                ]
                for k in range(ntiles):
                    sps = psS.tile([128, CPP * BCHUNK], F32)
                    for i in range(CPP):
                        c = p * CPP + i
                        nc.tensor.matmul(
                            sps[:, i * BCHUNK : (i + 1) * BCHUNK],
                            lhsT=kc_s[:, 128 * k : 128 * (k + 1)],
                            rhs=qc_s[:, BCHUNK * c : BCHUNK * (c + 1)],
                            start=True,
                            stop=True,
                        )
                    pT = pTp.tile([128, CPP * BCHUNK], BF16)
                    if k in dve_ks:
                        # crude-but-fast exp on VectorE: build bf16 bit pattern
                        nc.vector.tensor_scalar(
                            pT[:].bitcast(I16),
                            sps[:],
                            A_TRICK,
                            bias2_s[:, k : k + 1],
                            mybir.AluOpType.mult,
                            mybir.AluOpType.add,
                        )
                    else:
                        nc.scalar.activation(
                            pT[:],
                            sps[:],
                            mybir.ActivationFunctionType.Exp,
                            bias=bias_s[:, k : k + 1],
                            scale=1.0,
                        )
                    for i in range(CPP):
                        nc.tensor.matmul(
                            oaccs[i][:],
                            lhsT=va_s[:, VAW * k : VAW * (k + 1)],
                            rhs=pT[:, i * BCHUNK : (i + 1) * BCHUNK],
                            start=(k == 0),
                            stop=(k == ntiles - 1),
                        )
                for i in range(CPP):
                    c = p * CPP + i
                    nc.vector.tensor_copy(
                        oall[:, BCHUNK * c : BCHUNK * (c + 1)], oaccs[i][0 : D + 1, :]
                    )

                # per-pass partial out to DRAM; one merged ReduceScatter
                # covers the first NPASS-1 passes (hidden under the loop), a
                # second small one covers the last pass (the exposed tail)
                if p < NPASS - 1:
                    nc.sync.dma_start(
                        bounce3.ap()[:, p].rearrange("s p b -> p s b"),
                        oall[:, PASSB * p : PASSB * (p + 1)].rearrange(
                            "p (s b) -> p s b", s=NCORES
                        ),
                    )
                    if p == NPASS - 2:
                        nc.gpsimd.collective_compute(
                            "ReduceScatter",
                            mybir.AluOpType.add,
                            replica_groups=[list(range(NCORES))],
                            ins=[bounce3.ap().opt()],
                            outs=[red3.ap().opt()],
                        )
                else:
                    nc.sync.dma_start(
                        bounce1.ap().rearrange("s p b -> p s b"),
                        oall[:, PASSB * p : PASSB * (p + 1)].rearrange(
                            "p (s b) -> p s b", s=NCORES
                        ),
                    )
                    nc.gpsimd.collective_compute(
                        "ReduceScatter",
                        mybir.AluOpType.add,
                        replica_groups=[list(range(NCORES))],
                        ins=[bounce1.ap().opt()],
                        outs=[red1.ap().opt()],
                    )

            # finale: pull each pass's reduced share, transpose, divide, emit
            for p in range(NPASS):
                r_s = fin.tile([D + 1, SHARDB], F32, name="r_s", tag="rsred")
                if p < NPASS - 1:
                    nc.sync.dma_start(r_s[:], red3.ap()[p])
                else:
                    nc.sync.dma_start(r_s[:], red1.ap())
                tp = psS.tile(
                    [128, D + 1],
                    F32,
                    name="tp",
                    tag="sps",
                    padded_shape=[128, CPP * BCHUNK],
                )
                nc.tensor.transpose(
                    tp[:], in_=r_s[:], identity=ident[0 : D + 1, 0 : D + 1]
                )
                rcp = fin.tile([128, 1], F32, name="rcp", tag="rcp")
                nc.vector.reciprocal(rcp[:], tp[:, D : D + 1])
                ot = fin.tile([128, D], F32, name="ot", tag="ot")
                nc.scalar.activation(
                    ot[:],
                    tp[:, 0:D],
                    mybir.ActivationFunctionType.Copy,
                    bias=0.0,
                    scale=rcp[:],
                )
                nc.sync.dma_start(
                    out_ext.ap()[SHARDB * p : SHARDB * (p + 1), :], ot[:]
                )

    _skip_redundant_ldweights(nc)
    _split_multi_waits(nc)
    return nc


_BUILD_CACHE: dict[int, object] = {}


def kernel(
    query,
    context,
    mem_keys,
    mem_values,
    mem_contexts,
    mem_timestamps,
    used_slots,
    _want_trace: bool = False,
):
    query = np.asarray(query, dtype=np.float32)
    context = np.asarray(context, dtype=np.float32)
    mem_keys = np.asarray(mem_keys, dtype=np.float32)
    mem_values = np.asarray(mem_values, dtype=np.float32)
    mem_contexts = np.asarray(mem_contexts, dtype=np.float32)
    mem_timestamps = np.asarray(mem_timestamps, dtype=np.float32)
    used_slots = np.asarray(used_slots).astype(bool)

    idx = np.flatnonzero(used_slots)
    count = idx.size
    if count == 0:
        # softmax over uniformly -1e9 scores is uniform over all M slots
        return np.broadcast_to(
            mem_values.mean(axis=0, dtype=np.float64).astype(np.float32), (B, D)
        ).copy()

    m_loc = max(128, int(math.ceil(count / (NCORES * 128))) * 128)
    m_tot = m_loc * NCORES
    ntiles = m_loc // 128

    # host-side layout prep: compact used slots, pad, shard, fuse operands
    kc = np.zeros((m_tot, KPAD), dtype=np.float32)
    kc[:count, :D] = mem_keys[idx]
    kc[:count, D:KDIM] = mem_contexts[idx]
    va = np.zeros((m_tot, VAW), dtype=np.float32)
    va[:count, :D] = mem_values[idx]
    va[:, D] = 1.0
    ts = np.zeros(m_tot, dtype=np.float32)
    ts[:count] = mem_timestamps[idx]
    mk = np.zeros(m_tot, dtype=np.float32)
    mk[:count] = 1.0

    qc = np.zeros((B, KPAD), dtype=np.float32)
    qc[:, :D] = query
    qc[:, D:KDIM] = 0.5 * context
    qc_t = np.ascontiguousarray(qc.T).astype(ml_dtypes.bfloat16)

    in_maps = []
    for s in range(NCORES):
        lo, hi = s * m_loc, (s + 1) * m_loc
        va_tm = (
            va[lo:hi]
            .reshape(ntiles, 128, VAW)
            .transpose(1, 0, 2)
            .reshape(128, ntiles * VAW)
        )
        in_maps.append(
            {
                "qc_t": qc_t,
                "kc_t": np.ascontiguousarray(kc[lo:hi].T).astype(ml_dtypes.bfloat16),
                "vaug": np.ascontiguousarray(va_tm).astype(ml_dtypes.bfloat16),
                "tsm": np.ascontiguousarray(ts[lo:hi].reshape(ntiles, 128).T),
                "maskf": np.ascontiguousarray(mk[lo:hi].reshape(ntiles, 128).T),
            }
        )

    nc = _BUILD_CACHE.get(m_loc)
    if nc is None:
        nc = _build(m_loc)
        _BUILD_CACHE[m_loc] = nc

    res = bass_utils.run_bass_kernel_spmd(
        nc, in_maps, core_ids=list(range(NCORES)), trace=_want_trace
    )

    # reassemble: core i's output row (SHARDB*p + j) is batch row
    # (PASSB*p + SHARDB*i + j)
    out = np.empty((B, D), dtype=np.float32)
    for s in range(NCORES):
        o = res.results[s]["out"].reshape(NPASS, SHARDB, D)
        for p in range(NPASS):
            base = PASSB * p + SHARDB * s
            out[base : base + SHARDB] = o[p]
    if _want_trace:
        kernel.last_exec_time_ns = res.exec_time_ns
        kernel.last_results = res
    return out


# revision 14
# speedup vs baseline: 1.0144x; 1.0144x over previous
"""Trainium2 Bass kernel for nn_AdaptiveEpisodicMemory (scatter_memory).

Computes, for B=4096 queries over an M=65536-slot memory bank:

    scores = q @ K^T + 0.5 * c @ CTX^T + 0.3*exp(-0.1*(1-t))  (masked by used_slots)
    out    = softmax(scores) @ V

Strategy (8 NeuronCores):
  * Unused slots receive -1e9 scores; their softmax weight is exactly 0 in
    fp32, so the host drops them up-front (exact transformation) and pads the
    survivors to a multiple of 8*128. Shapes are chosen per-input at build
    time, so the kernel is correct for any input.
  * The memory bank (keys/contexts/values) is sharded across the 8 cores;
    query/context are replicated. Per core:
        S^T[m, b]  = KC_shard^T.T @ QC^T      (one K=128-padded matmul, bf16)
        P^T[m, b]  = exp(S^T + bias_m)        (ScalarE for most tiles; a bf16
                                               bit-trick exp on VectorE for a
                                               minority, to relieve ScalarE;
                                               bias folds time-decay + mask)
        O^T[65, b] += Vaug_tile.T @ P^T       (Vaug = [V | 1 | 0-pad]; row 64
                                               accumulates the softmax denom)
  * After each 1024-query pass the [65, 1024] partial is ReduceScattered
    (overlapped with the next pass); at the end each core transposes its
    4x128-query shares, divides by the denominator, and writes them out.
    Host work is limited to layout: compaction/sharding/concat/transpose of
    inputs and reassembly of output slices.
"""
import sys

sys.path.insert(0, "/opt/trn_rl_repo")
import math

import ml_dtypes
import numpy as np

from concourse import bass, bass_utils, masks, mybir, tile

B, M, D, CD = 4096, 65536, 64, 32
KDIM = D + CD  # 96: contraction dim of the fused score matmul
KPAD = 128  # padded to 128 so weight loads take the fast path
VAW = 128  # Vaug padded from 65 to 128 columns, same reason
NCORES = 8
BCHUNK = 512
CPP = 2  # batch chunks per pass (exp runs at FD = CPP*BCHUNK)
PASSB = BCHUNK * CPP
NPASS = B // PASSB
SHARDB = PASSB // NCORES  # batch rows each core owns per pass after RS
F32 = mybir.dt.float32
BF16 = mybir.dt.bfloat16
I16 = mybir.dt.int16
TIME_WEIGHT = 0.1
CURRENT_TIME = 1.0
DECAY_COEF = 0.3
NEG_BIG = -1e9
N_WARMUP_MM = 22
# bf16 bit-trick exp: bf16bits(e^x) ~ round(x * 128/ln2 + (127*128 - 5.5))
A_TRICK = 128.0 / math.log(2.0)
B_TRICK = 127.0 * 128.0 - 5.5
N_ACT_TAIL = 8  # trailing m-tiles always on ScalarE (they may hold -1e9 pads)
DVE_STRIDE = 3  # every 3rd eligible m-tile goes to VectorE


def _dve_ks(ntiles: int) -> set:
    return {k for k in range(max(0, ntiles - N_ACT_TAIL)) if k % DVE_STRIDE == 1}


def _split_multi_waits(nc) -> int:
    """This walrus build accepts at most one fused sync-wait per instruction;
    hoist extras into standalone InstEventSemaphore instructions."""
    n_split = 0
    for fn in nc.m.functions:
        for bb in fn.blocks:
            insts = list(bb.instructions)
            out = []
            changed = False
            for inst in insts:
                si = inst.sync_info
                if si is not None and si.on_wait is not None and len(si.on_wait) > 1:
                    waits = list(si.on_wait)
                    for w in waits[:-1]:
                        ev = mybir.InstEventSemaphore(
                            name=f"{inst.name}-wsplit{n_split}",
                            engine=inst.engine,
                            ins=[],
                            outs=[],
                            sync_info=mybir.SyncInfo(on_wait=[w], on_update=[]),
                            bass_nofuse=True,
                        )
                        out.append(ev)
                        n_split += 1
                    inst.sync_info = mybir.SyncInfo(
                        on_wait=[waits[-1]], on_update=list(si.on_update or [])
                    )
                    changed = True
                out.append(inst)
            if changed:
                bb.instructions[:] = out
    return n_split


def _skip_redundant_ldweights(nc) -> int:
    """After scheduling, matmuls whose stationary operand is identical to the
    immediately preceding matmul's can skip the implicit LDWEIGHTS."""
    n = 0
    for fn in nc.m.functions:
        for bb in fn.blocks:
            insts = list(bb.instructions)
            prev_w = None
            changed = False
            for inst in insts:
                if not isinstance(inst, mybir.InstMatmult):
                    continue
                if inst.is_transpose:
                    prev_w = None
                    continue
                w = repr(inst.ins[1])
                if prev_w is not None and w == prev_w:
                    inst.ldweights = False
                    changed = True
                    n += 1
                prev_w = w
            if changed:
                bb.instructions[:] = insts
    return n


def _build(m_loc: int):
    """Build the per-core Bass program for a shard of m_loc memory slots."""
    ntiles = m_loc // 128
    dve_ks = _dve_ks(ntiles)
    nc = bass.Bass(trn_type="TRN2", debug=False, num_devices=NCORES)

    # register the decay-exp bias as a const AP (only 0.0/1.0 are built in)
    decay_bias = math.log(DECAY_COEF) - TIME_WEIGHT * CURRENT_TIME
    ct = nc.alloc_sbuf_tensor("const-float32-extra", [128, 1], F32)
    nc.gpsimd.memset(ct.ap(), decay_bias)
    nc.const_aps.aps[(F32, decay_bias)] = ct.ap()
    nc.all_engine_barrier()

    qc_ext = nc.dram_tensor("qc_t", [KPAD, B], BF16, kind="ExternalInput")
    kc_ext = nc.dram_tensor("kc_t", [KPAD, m_loc], BF16, kind="ExternalInput")
    # vaug arrives pre-arranged tile-major: [128, ntiles*VAW]
    va_ext = nc.dram_tensor("vaug", [128, ntiles * VAW], BF16, kind="ExternalInput")
    ts_ext = nc.dram_tensor("tsm", [128, ntiles], F32, kind="ExternalInput")
    mk_ext = nc.dram_tensor("maskf", [128, ntiles], F32, kind="ExternalInput")
    out_ext = nc.dram_tensor("out", [B // NCORES, D], F32, kind="ExternalOutput")

    NP1 = NPASS - 1
    bounce3 = nc.dram_tensor("rs_in3", [NCORES, NP1, D + 1, SHARDB], F32)
    red3 = nc.dram_tensor("rs_out3", [NP1, D + 1, SHARDB], F32)
    bounce1 = nc.dram_tensor("rs_in1", [NCORES, D + 1, SHARDB], F32)
    red1 = nc.dram_tensor("rs_out1", [D + 1, SHARDB], F32)

    with tile.TileContext(nc) as tc:
        with (
            tc.tile_pool(name="big", bufs=1) as big,
            tc.tile_pool(name="small", bufs=1) as small,
            tc.tile_pool(name="pT", bufs=6) as pTp,
            tc.tile_pool(name="osb", bufs=1) as osb,
            tc.tile_pool(name="psS", bufs=3, space="PSUM") as psS,
            tc.tile_pool(name="psO", bufs=1, space="PSUM") as psO,
            tc.tile_pool(name="fin", bufs=2) as fin,
        ):
            # PE warmup: keep TensorE busy from t=0 so HAM reaches 2.4 GHz
            # before the real matmuls start (inputs are still DMAing in).
            wsrc = small.tile([128, 512], BF16)
            nc.vector.memset(wsrc[:], 1.0)
            wps = psS.tile(
                [128, 512], F32, name="wps", tag="sps", padded_shape=[128, CPP * BCHUNK]
            )
            for _ in range(N_WARMUP_MM):
                nc.tensor.matmul(
                    wps[:], lhsT=wsrc[:, 0:128], rhs=wsrc[:], start=True, stop=True
                )

            # small inputs first so the bias chain is ready early
            ts_s = small.tile([128, ntiles], F32)
            nc.sync.dma_start(ts_s[:], ts_ext.ap())
            mk_s = small.tile([128, ntiles], F32)
            nc.sync.dma_start(mk_s[:], mk_ext.ap())

            # big inputs, chunked; the first pieces of qc/kc come first so the
            # loop can start while the rest streams in
            qc_s = big.tile([KPAD, B], BF16)
            kc_s = big.tile([KPAD, m_loc], BF16)
            va_s = big.tile([128, ntiles * VAW], BF16)
            nkc = min(4, ntiles)
            wq, wk = B // NPASS, m_loc // nkc
            nc.sync.dma_start(qc_s[:, 0:wq], qc_ext.ap()[:, 0:wq])
            nc.sync.dma_start(kc_s[:, 0:wk], kc_ext.ap()[:, 0:wk])
            for c in range(1, nkc):
                nc.sync.dma_start(
                    kc_s[:, c * wk : (c + 1) * wk],
                    kc_ext.ap()[:, c * wk : (c + 1) * wk],
                )
            for c in range(1, NPASS):
                nc.sync.dma_start(
                    qc_s[:, c * wq : (c + 1) * wq],
                    qc_ext.ap()[:, c * wq : (c + 1) * wq],
                )
            for c in range(2):
                h = ntiles * VAW // 2
                nc.sync.dma_start(
                    va_s[:, c * h : (c + 1) * h], va_ext.ap()[:, c * h : (c + 1) * h]
                )

            # identity for the epilogue transposes (GpSimd, overlaps DMAs)
            ident = small.tile([128, 128], F32)
            masks.make_identity(nc, ident[:])

            # bias_m = 0.3*exp(0.1*t - 0.1) + (mask ? 0 : -1e9)
            #        = exp(0.1*t + (ln 0.3 - 0.1)) + (mask - 1) * 1e9
            d_s = small.tile([128, ntiles], F32)
            nc.scalar.activation(
                d_s[:],
                ts_s[:],
                mybir.ActivationFunctionType.Exp,
                bias=decay_bias,
                scale=TIME_WEIGHT,
            )
            mneg = small.tile([128, ntiles], F32)
            nc.vector.tensor_scalar(
                mneg[:],
                mk_s[:],
                -NEG_BIG,
                NEG_BIG,
                mybir.AluOpType.mult,
                mybir.AluOpType.add,
            )
            bias_s = small.tile([128, ntiles], F32)
            nc.vector.tensor_add(bias_s[:], d_s[:], mneg[:])
            # for the VectorE trick-exp tiles: bias2 = bias * A + B
            bias2_s = small.tile([128, ntiles], F32)
            nc.vector.tensor_scalar(
                bias2_s[:],
                bias_s[:],
                A_TRICK,
                B_TRICK,
                mybir.AluOpType.mult,
                mybir.AluOpType.add,
            )

            oall = osb.tile([D + 1, B], F32)

            for p in range(NPASS):
                oaccs = [
                    psO.tile([128, BCHUNK], F32, name=f"oacc{i}", tag=f"oacc{i}")
                    for i in range(CPP)
                ]
                for k in range(ntiles):
                    sps = psS.tile([128, CPP * BCHUNK], F32)
                    for i in range(CPP):
                        c = p * CPP + i
                        nc.tensor.matmul(
                            sps[:, i * BCHUNK : (i + 1) * BCHUNK],
                            lhsT=kc_s[:, 128 * k : 128 * (k + 1)],
                            rhs=qc_s[:, BCHUNK * c : BCHUNK * (c + 1)],
                            start=True,
                            stop=True,
                        )
                    pT = pTp.tile([128, CPP * BCHUNK], BF16)
                    if k in dve_ks:
                        # crude-but-fast exp on VectorE: build bf16 bit pattern
                        nc.vector.tensor_scalar(
                            pT[:].bitcast(I16),
                            sps[:],
                            A_TRICK,
                            bias2_s[:, k : k + 1],
                            mybir.AluOpType.mult,
                            mybir.AluOpType.add,
                        )
                    else:
                        nc.scalar.activation(
                            pT[:],
                            sps[:],
                            mybir.ActivationFunctionType.Exp,
                            bias=bias_s[:, k : k + 1],
                            scale=1.0,
                        )
                    for i in range(CPP):
                        nc.tensor.matmul(
                            oaccs[i][:],
                            lhsT=va_s[:, VAW * k : VAW * (k + 1)],
                            rhs=pT[:, i * BCHUNK : (i + 1) * BCHUNK],
                            start=(k == 0),
                            stop=(k == ntiles - 1),
                        )
                for i in range(CPP):
                    c = p * CPP + i
                    nc.vector.tensor_copy(
                        oall[:, BCHUNK * c : BCHUNK * (c + 1)], oaccs[i][0 : D + 1, :]
                    )

                # per-pass partial out to DRAM; one merged ReduceScatter
                # covers the first NPASS-1 passes (hidden under the loop), a
                # second small one covers the last pass (the exposed tail)
                if p < NPASS - 1:
                    nc.sync.dma_start(
                        bounce3.ap()[:, p].rearrange("s p b -> p s b"),
                        oall[:, PASSB * p : PASSB * (p + 1)].rearrange(
                            "p (s b) -> p s b", s=NCORES
                        ),
                    )
                    if p == NPASS - 2:
                        nc.gpsimd.collective_compute(
                            "ReduceScatter",
                            mybir.AluOpType.add,
                            replica_groups=[list(range(NCORES))],
                            ins=[bounce3.ap().opt()],
                            outs=[red3.ap().opt()],
                        )
                else:
                    nc.sync.dma_start(
                        bounce1.ap().rearrange("s p b -> p s b"),
                        oall[:, PASSB * p : PASSB * (p + 1)].rearrange(
                            "p (s b) -> p s b", s=NCORES
                        ),
                    )
                    nc.gpsimd.collective_compute(
                        "ReduceScatter",
                        mybir.AluOpType.add,
                        replica_groups=[list(range(NCORES))],
                        ins=[bounce1.ap().opt()],
                        outs=[red1.ap().opt()],
                    )

            # finale: pull each pass's reduced share, transpose, divide, emit
            for p in range(NPASS):
                r_s = fin.tile([D + 1, SHARDB], F32, name="r_s", tag="rsred")
                if p < NPASS - 1:
                    nc.sync.dma_start(r_s[:], red3.ap()[p])
                else:
                    nc.sync.dma_start(r_s[:], red1.ap())
                tp = psS.tile(
                    [128, D + 1],
                    F32,
                    name="tp",
                    tag="sps",
                    padded_shape=[128, CPP * BCHUNK],
                )
                nc.tensor.transpose(
                    tp[:], in_=r_s[:], identity=ident[0 : D + 1, 0 : D + 1]
                )
                rcp = fin.tile([128, 1], F32, name="rcp", tag="rcp")
                nc.vector.reciprocal(rcp[:], tp[:, D : D + 1])
                ot = fin.tile([128, D], F32, name="ot", tag="ot")
                nc.scalar.activation(
                    ot[:],
                    tp[:, 0:D],
                    mybir.ActivationFunctionType.Copy,
                    bias=0.0,
                    scale=rcp[:],
                )
                nc.sync.dma_start(
                    out_ext.ap()[SHARDB * p : SHARDB * (p + 1), :], ot[:]
                )

    _skip_redundant_ldweights(nc)
    _split_multi_waits(nc)
    return nc


_BUILD_CACHE: dict[int, object] = {}


def kernel(
    query,
    context,
    mem_keys,
    mem_values,
    mem_contexts,
    mem_timestamps,
    used_slots,
    _want_trace: bool = False,
):
    query = np.asarray(query, dtype=np.float32)
    context = np.asarray(context, dtype=np.float32)
    mem_keys = np.asarray(mem_keys, dtype=np.float32)
    mem_values = np.asarray(mem_values, dtype=np.float32)
    mem_contexts = np.asarray(mem_contexts, dtype=np.float32)
    mem_timestamps = np.asarray(mem_timestamps, dtype=np.float32)
    used_slots = np.asarray(used_slots).astype(bool)

    idx = np.flatnonzero(used_slots)
    count = idx.size
    if count == 0:
        # softmax over uniformly -1e9 scores is uniform over all M slots
        return np.broadcast_to(
            mem_values.mean(axis=0, dtype=np.float64).astype(np.float32), (B, D)
        ).copy()

    m_loc = max(128, int(math.ceil(count / (NCORES * 128))) * 128)
    m_tot = m_loc * NCORES
    ntiles = m_loc // 128

    # host-side layout prep: compact used slots, pad, shard, fuse operands
    kc = np.zeros((m_tot, KPAD), dtype=np.float32)
    kc[:count, :D] = mem_keys[idx]
    kc[:count, D:KDIM] = mem_contexts[idx]
    va = np.zeros((m_tot, VAW), dtype=np.float32)
    va[:count, :D] = mem_values[idx]
    va[:, D] = 1.0
    ts = np.zeros(m_tot, dtype=np.float32)
    ts[:count] = mem_timestamps[idx]
    mk = np.zeros(m_tot, dtype=np.float32)
    mk[:count] = 1.0

    qc = np.zeros((B, KPAD), dtype=np.float32)
    qc[:, :D] = query
    qc[:, D:KDIM] = 0.5 * context
    qc_t = np.ascontiguousarray(qc.T).astype(ml_dtypes.bfloat16)

    in_maps = []
    for s in range(NCORES):
        lo, hi = s * m_loc, (s + 1) * m_loc
        va_tm = (
            va[lo:hi]
            .reshape(ntiles, 128, VAW)
            .transpose(1, 0, 2)
            .reshape(128, ntiles * VAW)
        )
        in_maps.append(
            {
                "qc_t": qc_t,
                "kc_t": np.ascontiguousarray(kc[lo:hi].T).astype(ml_dtypes.bfloat16),
                "vaug": np.ascontiguousarray(va_tm).astype(ml_dtypes.bfloat16),
                "tsm": np.ascontiguousarray(ts[lo:hi].reshape(ntiles, 128).T),
                "maskf": np.ascontiguousarray(mk[lo:hi].reshape(ntiles, 128).T),
            }
        )

    nc = _BUILD_CACHE.get(m_loc)
    if nc is None:
        nc = _build(m_loc)
        _BUILD_CACHE[m_loc] = nc

    res = bass_utils.run_bass_kernel_spmd(
        nc, in_maps, core_ids=list(range(NCORES)), trace=_want_trace
    )

    # reassemble: core i's output row (SHARDB*p + j) is batch row
    # (PASSB*p + SHARDB*i + j)
    out = np.empty((B, D), dtype=np.float32)
    for s in range(NCORES):
        o = res.results[s]["out"].reshape(NPASS, SHARDB, D)
        for p in range(NPASS):
            base = PASSB * p + SHARDB * s
            out[base : base + SHARDB] = o[p]
    if _want_trace:
        kernel.last_exec_time_ns = res.exec_time_ns
        kernel.last_results = res
    return out
